# revision 5
# baseline (speedup 1.0000x reference)
"""Multi-head attention (BS=4, SL=2048, D=1024, H=16) on 8 TRN2 NeuronCores.

Sharding: batch x head-half. Core c handles batch c//2, heads (c%2)*8..(c%2)*8+8.
Each core computes its 8 heads end-to-end (QKV projections restricted to its
512-wide feature slice, attention, and a partial output projection); the host
sums the two partial output projections per batch and adds the output bias.

Layout strategy (per core):
  - Host uploads x^T (feature-major) with a ones row appended, and weight
    slices pre-transposed to [in, out] with the bias appended as an extra
    input row, so every projection including bias is pure matmul.
  - Q and K are produced transposed ([feature, token]) which is exactly the
    lhsT/rhs layout the attention score matmul wants; V is produced
    token-major with a ones column per head, so the PV matmul's row 64 yields
    the softmax denominator for free.
  - Scores are computed transposed (S^T[key, query]); softmax reduction over
    keys becomes the PE contraction; exp runs on the scalar engine straight
    out of PSUM; attention output appears transposed, which is exactly the
    lhsT layout the output projection wants. No on-device transposes anywhere.
  - All matmuls run as float32r (1 cycle/row at N>=256, ~1.6e-4 rel err).
"""

import numpy as np

BS, SL, D, H, HD = 4, 2048, 1024, 16, 64
NCORES = 8
HPC = H // 2          # heads per core = 8
OF = HPC * HD         # per-core feature slice = 512
KC = D // 128         # 8 contraction chunks of 128 (+1 bias row chunk)
IB = 1024             # query-block width for attention
NIB = SL // IB        # 2
SCALE = 1.0 / np.sqrt(HD)

_CACHE = {}


def _build_nc():
    if "nc" in _CACHE:
        return _CACHE["nc"]
    import concourse.bass as bass  # noqa: F401
    from concourse import bacc
    import concourse.mybir as mybir
    import concourse.tile as tile

    f32 = mybir.dt.float32
    f32r = mybir.dt.float32r
    EXP = mybir.ActivationFunctionType.Exp

    nc = bacc.Bacc("TRN2", target_bir_lowering=False, debug=False,
                   num_devices=NCORES)

    xqT = nc.declare_dram_parameter("xqT", [D + 1, SL], f32, isOutput=False)
    xkT = nc.declare_dram_parameter("xkT", [D + 1, SL], f32, isOutput=False)
    xvT = nc.declare_dram_parameter("xvT", [D + 1, SL], f32, isOutput=False)
    wqT = nc.declare_dram_parameter("wqT", [D + 1, OF], f32, isOutput=False)
    wkT = nc.declare_dram_parameter("wkT", [D + 1, OF], f32, isOutput=False)
    wvT = nc.declare_dram_parameter("wvT", [D + 1, OF + HPC], f32, isOutput=False)
    woT = nc.declare_dram_parameter("woT", [OF, D], f32, isOutput=False)
    out = nc.declare_dram_parameter("out", [SL, D], f32, isOutput=True)

    VW = OF + HPC  # 520: V plus one ones-column per head

    with tile.TileContext(nc) as tc:
        with (
            tc.tile_pool(name="qkv", bufs=1) as qkv,
            tc.tile_pool(name="pt", bufs=2) as ptp,
            tc.tile_pool(name="norm", bufs=1) as nrm,
        ):
            qt = [qkv.tile([128, SL], f32r, name=f"qt{i}", tag=f"qt{i}") for i in range(4)]
            kt = [qkv.tile([128, SL], f32r, name=f"kt{i}", tag=f"kt{i}") for i in range(4)]
            vs = [qkv.tile([128, VW], f32r, name=f"v{i}", tag=f"v{i}") for i in range(16)]

            # ---- projections -------------------------------------------
            def proj_qk(x_dram, w_dram, dst):
                with (
                    tc.tile_pool(name="w", bufs=1) as wp,
                    tc.tile_pool(name="x", bufs=1) as xp,
                    tc.tile_pool(name="ps", bufs=4, space="PSUM") as ps,
                ):
                    w = [wp.tile([128, OF], f32r, name=f"w{k}", tag=f"w{k}") for k in range(KC)]
                    w.append(wp.tile([1, OF], f32r, name="w8", tag="w8"))
                    for k in range(KC):
                        nc.sync.dma_start(
                            out=w[k][:], in_=w_dram[k * 128:(k + 1) * 128, :].bitcast(f32r))
                    nc.sync.dma_start(out=w[KC][:], in_=w_dram[D:D + 1, :].bitcast(f32r))
                    for tb in range(4):  # token blocks of 512
                        x = [xp.tile([128, 512], f32r, name=f"x{k}", tag=f"x{k}") for k in range(KC)]
                        x.append(xp.tile([1, 512], f32r, name="x8", tag="x8"))
                        for k in range(KC):
                            nc.sync.dma_start(
                                out=x[k][:],
                                in_=x_dram[k * 128:(k + 1) * 128,
                                           tb * 512:(tb + 1) * 512].bitcast(f32r))
                        nc.sync.dma_start(
                            out=x[KC][:],
                            in_=x_dram[D:D + 1, tb * 512:(tb + 1) * 512].bitcast(f32r))
                        for of_t in range(4):
                            p = ps.tile([128, 512], f32, name="p", tag="p")
                            for k in range(KC + 1):
                                nc.tensor.matmul(
                                    p[:], w[k][:, of_t * 128:(of_t + 1) * 128],
                                    x[k][:], start=(k == 0), stop=(k == KC))
                            nc.vector.tensor_copy(
                                dst[of_t][:, tb * 512:(tb + 1) * 512], p[:])

            def proj_v():
                with (
                    tc.tile_pool(name="w", bufs=1) as wp,
                    tc.tile_pool(name="x", bufs=1) as xp,
                    tc.tile_pool(name="ps", bufs=2, space="PSUM") as ps,
                ):
                    w = [wp.tile([128, VW], f32r, name=f"w{k}", tag=f"w{k}") for k in range(KC)]
                    w.append(wp.tile([1, VW], f32r, name="w8", tag="w8"))
                    for k in range(KC):
                        nc.sync.dma_start(
                            out=w[k][:], in_=wvT[k * 128:(k + 1) * 128, :].bitcast(f32r))
                    nc.sync.dma_start(out=w[KC][:], in_=wvT[D:D + 1, :].bitcast(f32r))
                    for tb in range(4):
                        x = [xp.tile([128, 512], f32r, name=f"x{k}", tag=f"x{k}") for k in range(KC)]
                        x.append(xp.tile([1, 512], f32r, name="x8", tag="x8"))
                        for k in range(KC):
                            nc.sync.dma_start(
                                out=x[k][:],
                                in_=xvT[k * 128:(k + 1) * 128,
                                        tb * 512:(tb + 1) * 512].bitcast(f32r))
                        nc.sync.dma_start(
                            out=x[KC][:],
                            in_=xvT[D:D + 1, tb * 512:(tb + 1) * 512].bitcast(f32r))
                        for ts in range(4):  # token tiles of 128 within block
                            tt = tb * 4 + ts
                            pa = ps.tile([128, 512], f32, name="pa", tag="pa")
                            pb = ps.tile([128, 8], f32, name="pb", tag="pb")
                            for k in range(KC + 1):
                                lhs = x[k][:, ts * 128:(ts + 1) * 128]
                                nc.tensor.matmul(pa[:], lhs, w[k][:, 0:512],
                                                 start=(k == 0), stop=(k == KC))
                                nc.tensor.matmul(pb[:], lhs, w[k][:, 512:VW],
                                                 start=(k == 0), stop=(k == KC))
                            nc.vector.tensor_copy(vs[tt][:, 0:512], pa[:])
                            nc.vector.tensor_copy(vs[tt][:, 512:VW], pb[:])

            proj_v()
            proj_qk(xkT, wkT, kt)
            proj_qk(xqT, wqT, qt)

            # ---- attention ---------------------------------------------
            otp_cm = tc.tile_pool(name="otp", bufs=1)
            otp = otp_cm.__enter__()
            ot = [otp.tile([128, SL], f32r, name=f"ot{i}", tag=f"ot{i}")
                  for i in range(4)]
            with tc.tile_pool(name="ps_att", bufs=1, space="PSUM") as psa:
                for ib in range(NIB):
                    isl = slice(ib * IB, (ib + 1) * IB)
                    for hp in range(4):
                        hA, hB = 2 * hp, 2 * hp + 1
                        oA = psa.tile([65, IB], f32, name="oA", tag="oA")
                        oB = psa.tile([65, IB], f32, name="oB", tag="oB")
                        for j in range(16):
                            jsl = slice(j * 128, (j + 1) * 128)
                            sA = psa.tile([128, IB], f32, name="sA", tag="sA")
                            sB = psa.tile([128, IB], f32, name="sB", tag="sB")
                            for nb in range(IB // 512):
                                nsl = slice(nb * 512, (nb + 1) * 512)
                                qsl = slice(ib * IB + nb * 512, ib * IB + nb * 512 + 512)
                                nc.tensor.matmul(
                                    sA[:, nsl], kt[hp][0:64, jsl], qt[hp][0:64, qsl],
                                    start=True, stop=True, tile_position=(0, 0))
                                nc.tensor.matmul(
                                    sB[:, nsl], kt[hp][64:128, jsl], qt[hp][64:128, qsl],
                                    start=True, stop=True, tile_position=(64, 0))
                            pA = ptp.tile([128, IB], f32r, name="ptA", tag="ptA")
                            pB = ptp.tile([128, IB], f32r, name="ptB", tag="ptB")
                            nc.scalar.activation(pA[:], sA[:], EXP, scale=float(SCALE))
                            nc.scalar.activation(pB[:], sB[:], EXP, scale=float(SCALE))
                            vA = vs[j][:, hA * 65:hA * 65 + 65]
                            vB = vs[j][:, hB * 65:hB * 65 + 65]
                            for nb in range(IB // 512):
                                nsl = slice(nb * 512, (nb + 1) * 512)
                                nc.tensor.matmul(oA[:, nsl], vA, pA[:, nsl],
                                                 start=(j == 0), stop=(j == 15))
                                nc.tensor.matmul(oB[:, nsl], vB, pB[:, nsl],
                                                 start=(j == 0), stop=(j == 15))
                        for head, o_ps in ((0, oA), (1, oB)):
                            r0 = nrm.tile([1, IB], f32, name="r0", tag="r0")
                            r1 = nrm.tile([1, IB], f32, name="r1", tag="r1")
                            bc = nrm.tile([64, IB], f32, name="bc", tag="bc")
                            nc.vector.tensor_copy(r0[:], o_ps[64:65, :])
                            nc.vector.reciprocal_approx_fast(r1[:], r0[:])
                            nc.gpsimd.partition_broadcast(bc[:], r1[0:1, :])
                            nc.vector.tensor_mul(
                                ot[hp][head * 64:head * 64 + 64, isl],
                                o_ps[0:64, :], bc[:])

            # ---- output projection -------------------------------------
            with (
                tc.tile_pool(name="wo", bufs=1) as wop,
                tc.tile_pool(name="ob", bufs=4) as obp,
                tc.tile_pool(name="ps_o", bufs=3, space="PSUM") as pso,
            ):
                wo = [wop.tile([128, D], f32r, name=f"wo{i}", tag=f"wo{i}") for i in range(4)]
                for oc in range(4):
                    nc.sync.dma_start(
                        out=wo[oc][:], in_=woT[oc * 128:(oc + 1) * 128, :].bitcast(f32r))
                for tt in range(16):
                    tsl = slice(tt * 128, (tt + 1) * 128)
                    for nb in range(2):
                        nsl = slice(nb * 512, (nb + 1) * 512)
                        p = pso.tile([128, 512], f32, name="p", tag="p")
                        for oc in range(4):
                            nc.tensor.matmul(p[:], ot[oc][:, tsl], wo[oc][:, nsl],
                                             start=(oc == 0), stop=(oc == 3))
                        ob = obp.tile([128, 512], f32, name="ob", tag="ob")
                        nc.vector.tensor_copy(ob[:], p[:])
                        nc.sync.dma_start(out=out[tsl, nsl], in_=ob[:])
            otp_cm.__exit__(None, None, None)

    nc.compile()
    _CACHE["nc"] = nc
    return nc


def _host_prep(value, key_in, query, Wq, bq, Wk, bk, Wv, bv, Wo, bo):
    """Build per-core input maps. Batch tensors are shared across core pairs."""
    f32 = np.float32

    def x_ext(x_b):  # [SL, D] -> [D+1, SL] with ones row
        r = np.empty((D + 1, SL), f32)
        r[:D] = np.ascontiguousarray(x_b.T)
        r[D] = 1.0
        return r

    def w_ext(W, b, g):  # [D+1, OF] slice for half g
        r = np.empty((D + 1, OF), f32)
        sl = slice(g * OF, (g + 1) * OF)
        r[:D] = np.ascontiguousarray(W[sl, :].T)
        r[D] = b[sl]
        return r

    def wv_ext(g):  # [D+1, OF+HPC]: per head 64 cols of Wv^T then a ones col
        r = np.zeros((D + 1, OF + HPC), f32)
        for h in range(HPC):
            of = g * OF + h * HD
            c = h * (HD + 1)
            r[:D, c:c + HD] = Wv[of:of + HD, :].T
            r[D, c:c + HD] = bv[of:of + HD]
            r[D, c + HD] = 1.0
        return r

    xq = [x_ext(np.asarray(query[b], f32)) for b in range(BS)]
    xk = [x_ext(np.asarray(key_in[b], f32)) for b in range(BS)]
    xv = [x_ext(np.asarray(value[b], f32)) for b in range(BS)]
    wq = [w_ext(np.asarray(Wq, f32), np.asarray(bq, f32), g) for g in range(2)]
    wk = [w_ext(np.asarray(Wk, f32), np.asarray(bk, f32), g) for g in range(2)]
    Wv = np.asarray(Wv, f32)
    bv = np.asarray(bv, f32)
    wv = [wv_ext(g) for g in range(2)]
    WoT = np.ascontiguousarray(np.asarray(Wo, f32).T)  # [in, out]
    wo = [np.ascontiguousarray(WoT[g * OF:(g + 1) * OF, :]) for g in range(2)]

    in_maps = []
    for c in range(NCORES):
        b, g = c // 2, c % 2
        in_maps.append({
            "xqT": xq[b], "xkT": xk[b], "xvT": xv[b],
            "wqT": wq[g], "wkT": wk[g], "wvT": wv[g], "woT": wo[g],
        })
    return in_maps


LAST_EXEC_NS = None
LAST_RESULTS = None


def kernel(value, key_in, query, Wq, bq, Wk, bk, Wv, bv, Wo, bo):
    import os
    from concourse.bass_utils import run_bass_kernel_spmd

    global LAST_EXEC_NS, LAST_RESULTS
    nc = _build_nc()
    in_maps = _host_prep(value, key_in, query, Wq, bq, Wk, bk, Wv, bv, Wo, bo)
    res = run_bass_kernel_spmd(nc, in_maps, list(range(NCORES)),
                               trace=bool(os.environ.get("BASS_TRACE")))
    LAST_EXEC_NS = res.exec_time_ns
    LAST_RESULTS = res
    bo = np.asarray(bo, np.float32)
    o = np.empty((BS, SL, D), np.float32)
    for b in range(BS):
        o[b] = res.results[2 * b]["out"] + res.results[2 * b + 1]["out"] + bo
    return o
